# revision 2
# baseline (speedup 1.0000x reference)
"""LlamaAttention (B=1,S=2048,D=4096,NH=32,NKV=8,HD=128) on 8 TRN2 NeuronCores.

Sharding: tensor-parallel over heads (4 Q heads + 1 KV head per core).
Everything on-device runs in a transposed [feature, seq] layout so no PE
transposes are needed anywhere:
  - host ships x^T, wqkv^T-shard, wo^T-shard, cos^T/sin^T as bf16
  - QKV projection produces Q^T/K^T directly; V is produced in natural
    [seq, hd] layout (it is the AV matmul's stationary operand)
  - scores_T[k,q] = (K^T)^T . Q^T per 128x512 tile; exp on ACT engine
  - softmax denominator: exp tiles pair-summed on GpSimd, then a
    ones-vector matmul chain on PE reduces over partitions
  - y^T accumulated in PSUM, normalized with a partition-broadcast
    reciprocal (rank-1 ones outer product on PE)
  - wo is ROW-PARALLEL (Megatron style): each core contracts its own 4
    heads' y^T straight out of SBUF into a partial out^T[4096e, 512s]
    per q-block -- no gather, no DRAM round trip of y
  - a per-q-block ReduceScatter(add) sums the partials and lands each
    core's 512-col e-shard; no compute ever waits on a collective (the
    only post-collective op is a tiny DRAM->DRAM hop into the output)
  - host transposes/concatenates the 8 transposed column shards
All bulk HBM traffic uses batched 3D-access-pattern DMAs (one descriptor
per multi-tile panel) to keep the DMA-trigger sequencers off the
critical path, spread across the SP/ACT/DVE trigger queues.
Mask handling is chosen host-side: causal fast path (skip upper-tri
k-tiles, additive diagonal patterns), all-zeros path (no mask at all), or
general path (stream mask^T/scale tiles and add before exp).
"""

import os
import sys
from contextlib import ExitStack

sys.path.insert(0, "/opt/trn_rl_repo")

import ml_dtypes
import numpy as np

import concourse.bass as bass
import concourse.mybir as mybir
import concourse.tile as tile
from concourse import bacc, bass_utils

F32 = mybir.dt.float32
BF16 = mybir.dt.bfloat16

B, S, D = 1, 2048, 4096
NH, NKV, HD = 32, 8, 128
NCORES = 8
QH = NH // NCORES            # 4 Q heads per core
EQK = QH * HD + HD           # 640 cols of wqkT per core (4 Q heads + 1 K head)
ESH = D // NCORES            # 512 output cols per core
SCALE = 1.0 / float(np.sqrt(HD))
NEG = -1e9

SB = 512                     # seq block (matmul free dim)
NSB = S // SB                # 4
NKT = S // 128               # 16 k tiles
NDC = D // 128               # 32 contraction chunks

LAST_RESULT = None           # BassKernelResults of the most recent run


def _bf16(a):
    return np.ascontiguousarray(a).astype(ml_dtypes.bfloat16)


def _build_program(mask_mode: str, reps: int = 1) -> bass.Bass:
    if mask_mode == "general":
        return _build_program_general()

    causal = mask_mode == "causal"
    nc = bacc.Bacc(target_bir_lowering=False, trn_type="TRN2")

    xT = nc.declare_dram_parameter("xT", [D, S], BF16, isOutput=False)
    wqkT = nc.declare_dram_parameter("wqkT", [D, EQK], BF16, isOutput=False)
    wvT = nc.declare_dram_parameter("wvT", [D, HD], BF16, isOutput=False)
    # row-parallel wo: this core's 512 head-dims x all 4096 output cols
    woT = nc.declare_dram_parameter("woT", [ESH, D], BF16, isOutput=False)
    cosT = nc.declare_dram_parameter("cosT", [HD, S], BF16, isOutput=False)
    sinT = nc.declare_dram_parameter("sinT", [HD, S], BF16, isOutput=False)
    if causal:
        diagp = nc.declare_dram_parameter("diagp", [128, 4 * SB], BF16, isOutput=False)
    # out^T shard per q-block, written DIRECTLY by the ReduceScatter
    out = nc.declare_dram_parameter("out", [NSB, ESH, SB], BF16, isOutput=True)

    with tile.TileContext(nc) as tc, ExitStack() as ctx:
        persist = ctx.enter_context(tc.tile_pool(name="persist", bufs=1))
        dram = ctx.enter_context(tc.tile_pool(name="dram", bufs=1, space="DRAM"))

        def make_rs_tiles(rep):
            sfx = f"_r{rep}" if rep else ""
            rsi = [[dram.tile([D // 2, SB], BF16, name=f"rsin{q}{hf}{sfx}",
                              tag=f"rsin{q}{hf}{sfx}") for hf in "ab"]
                   for q in range(NSB)]
            rso = [[dram.tile([ESH // 2, SB], BF16, name=f"rsout{q}{hf}{sfx}",
                              tag=f"rsout{q}{hf}{sfx}") for hf in "ab"]
                   for q in range(NSB)]
            return rsi, rso

        rsin, rsout = make_rs_tiles(0)

        # ---- resident weights / tables (batched panel DMAs on SP) ------
        # first wqk dc-chunks and the first x panel arrive in small pieces
        # so the first QKV chain starts within a few microseconds
        wqk_big = persist.tile([128, NDC * EQK], BF16, name="wqk", tag="wqk")
        xpool = ctx.enter_context(tc.tile_pool(name="x", bufs=2))

        def load_x_half(sb, half, pieces=1):
            t = xpool.tile([128, 16 * SB], BF16, name="xh", tag="xh")
            for pc in range(pieces):
                w = 16 // pieces
                nc.sync.dma_start(
                    out=t[:, pc * w * SB:(pc + 1) * w * SB]
                        .rearrange("p (i c) -> p i c", i=w),
                    in_=xT[half * 2048 + pc * w * 128:
                           half * 2048 + (pc + 1) * w * 128,
                           sb * SB:(sb + 1) * SB]
                        .rearrange("(i p) c -> p i c", p=128))
            return t

        def load_wqk(g, n, pieces=1):
            for pc in range(pieces):
                w = n // pieces
                g0 = g + pc * w
                nc.sync.dma_start(
                    out=wqk_big[:, g0 * EQK:(g0 + w) * EQK]
                        .rearrange("p (i e) -> p i e", i=w),
                    in_=wqkT[g0 * 128:(g0 + w) * 128, :]
                        .rearrange("(i p) e -> p i e", p=128))

        load_wqk(0, 4, pieces=2)
        x00 = load_x_half(0, 0, pieces=8)
        load_wqk(4, 4, pieces=2)
        load_wqk(8, 8, pieces=2)
        x01 = load_x_half(0, 1, pieces=4)
        load_wqk(16, 16, pieces=2)
        first_x = [x00, x01]

        wv_big = persist.tile([128, NDC * HD], BF16, name="wv", tag="wv")
        nc.sync.dma_start(
            out=wv_big[:].rearrange("p (i e) -> p i e", i=NDC),
            in_=wvT[:].rearrange("(i p) e -> p i e", p=128))
        cos_sb = persist.tile([HD, S], BF16, name="cos", tag="cos")
        nc.sync.dma_start(out=cos_sb[:], in_=cosT[:, :])
        sin_sb = persist.tile([HD, S], BF16, name="sin", tag="sin")
        nc.sync.dma_start(out=sin_sb[:], in_=sinT[:, :])
        if causal:
            diag_sb = persist.tile([128, 4 * SB], BF16, name="diag", tag="diag")
            nc.sync.dma_start(out=diag_sb[:], in_=diagp[:, :])
        # wo panel: 4 local-head d-chunks x 4096 e cols
        wo_big = persist.tile([128, QH * D], BF16, name="wo", tag="wo")
        nc.sync.dma_start(
            out=wo_big[:].rearrange("p (i e) -> p i e", i=QH),
            in_=woT[:].rearrange("(i p) e -> p i e", p=128))

        ones_bf = persist.tile([128, 1], BF16, name="ones_bf", tag="ones_bf")
        nc.vector.memset(ones_bf[:], 1.0)
        ones_row = persist.tile([1, 128], BF16, name="ones_row", tag="ones_row")
        nc.vector.memset(ones_row[:], 1.0)

        # persistent activations: Q^T per head + K^T (RoPE'd in place), V
        qkT = [persist.tile([HD, S], BF16, name=f"qkT{e}", tag=f"qkT{e}")
               for e in range(QH + 1)]
        v_sb = persist.tile([128, S], BF16, name="v", tag="v")

        qkvps = ctx.enter_context(tc.tile_pool(name="qkvps", bufs=2, space="PSUM"))
        spool = ctx.enter_context(tc.tile_pool(name="sc_ps", bufs=3, space="PSUM"))
        ypool = ctx.enter_context(tc.tile_pool(name="y_ps", bufs=1, space="PSUM"))
        dpool = ctx.enter_context(tc.tile_pool(name="d_ps", bufs=1, space="PSUM"))
        rtmp = ctx.enter_context(tc.tile_pool(name="rtmp", bufs=2))
        epool = ctx.enter_context(tc.tile_pool(name="exp", bufs=6))
        ppool = ctx.enter_context(tc.tile_pool(name="pair", bufs=2))
        opool = ctx.enter_context(tc.tile_pool(name="attout", bufs=2))
        ynpool = ctx.enter_context(tc.tile_pool(name="ynorm", bufs=5))
        rspool = ctx.enter_context(tc.tile_pool(name="rsp", bufs=1))

        H2 = HD // 2

        def qkv_block(sb, xh):
            # Q^T / K^T: 5 chains of 32 matmuls, FD=512
            for et in range(QH + 1):
                ps = qkvps.tile([128, SB], F32, name="ps", tag="ps")
                for dc in range(NDC):
                    nc.tensor.matmul(
                        ps[:],
                        lhsT=wqk_big[:, dc * EQK + et * 128:dc * EQK + (et + 1) * 128],
                        rhs=xh[dc // 16][:, (dc % 16) * SB:(dc % 16 + 1) * SB],
                        start=(dc == 0), stop=(dc == NDC - 1))
                nc.scalar.copy(qkT[et][:, sb * SB:(sb + 1) * SB], ps[:])
            # V natural: 4 chains of 32 matmuls (FD=128) into free-dim
            # slices of one shared psum tile, drained with a single copy
            psv = qkvps.tile([128, SB], F32, name="ps", tag="ps")
            for st in range(SB // 128):
                for dc in range(NDC):
                    nc.tensor.matmul(
                        psv[:, st * 128:(st + 1) * 128],
                        lhsT=xh[dc // 16][:, (dc % 16) * SB + st * 128:
                                          (dc % 16) * SB + (st + 1) * 128],
                        rhs=wv_big[:, dc * HD:(dc + 1) * HD],
                        start=(dc == 0), stop=(dc == NDC - 1))
            nc.scalar.copy(v_sb[:, sb * SB:(sb + 1) * SB], psv[:])
            # RoPE in place on the 5 fresh [HD, SB] slices (DVE)
            sl = slice(sb * SB, (sb + 1) * SB)
            for et in range(QH + 1):
                src = qkT[et]
                rot = rtmp.tile([128, SB], BF16, name="rot", tag="rot")
                nc.vector.tensor_copy(rot[0:H2, :], src[H2:HD, sl])
                nc.vector.tensor_copy(rot[H2:HD, :], src[0:H2, sl])
                t1 = rtmp.tile([128, SB], BF16, name="t1", tag="t1")
                nc.vector.tensor_tensor(
                    t1[:], src[:, sl], cos_sb[:, sl], mybir.AluOpType.mult)
                t2 = rtmp.tile([128, SB], BF16, name="t2", tag="t2")
                nc.vector.tensor_tensor(
                    t2[:], rot[:], sin_sb[:, sl], mybir.AluOpType.mult)
                nc.vector.tensor_tensor(
                    src[:, sl], t1[:], t2[:], mybir.AluOpType.add)

        def attention_block(qb):
            qsl = slice(qb * SB, (qb + 1) * SB)
            klim = (qb + 1) * (SB // 128) if causal else NKT
            ynorms = []
            for h in range(QH):
                ps_y = ypool.tile([HD, SB], F32, name="psy", tag="psy")
                ps_d = dpool.tile([1, SB], F32, name="psd", tag="psd")
                et_prev = None
                for kt in range(klim):
                    ps_s = spool.tile([128, SB], F32, name="pss", tag="pss")
                    nc.tensor.matmul(
                        ps_s[:],
                        lhsT=qkT[QH][:, kt * 128:(kt + 1) * 128],
                        rhs=qkT[h][:, qsl],
                        start=True, stop=True)
                    et = epool.tile([128, SB], BF16, name="et", tag="et")
                    if causal and kt >= qb * (SB // 128):
                        j = kt - qb * (SB // 128)
                        nc.vector.tensor_tensor(
                            ps_s[:], ps_s[:],
                            diag_sb[:, j * SB:(j + 1) * SB],
                            mybir.AluOpType.add)
                    nc.scalar.activation(
                        et[:], ps_s[:],
                        mybir.ActivationFunctionType.Exp, scale=SCALE)
                    nc.tensor.matmul(
                        ps_y[:],
                        lhsT=v_sb[:, kt * 128:(kt + 1) * 128],
                        rhs=et[:],
                        start=(kt == 0), stop=(kt == klim - 1))
                    # denominator: pair-sum on GpSimd, reduce pairs on PE
                    if kt % 2 == 0:
                        et_prev = et
                    else:
                        pr = ppool.tile([128, SB], BF16, name="pr", tag="pr")
                        nc.gpsimd.tensor_tensor(
                            pr[:], et_prev[:], et[:], mybir.AluOpType.add)
                        nc.tensor.matmul(
                            ps_d[:], lhsT=ones_bf[:], rhs=pr[:],
                            start=(kt == 1), stop=(kt == klim - 1))
                recip = opool.tile([1, SB], BF16, name="recip", tag="recip")
                with nc.allow_low_precision(
                        reason="softmax denom is positive and O(100); bf16 "
                               "reciprocal feeds a bf16 broadcast anyway"):
                    nc.vector.reciprocal(recip[:], ps_d[:])
                # broadcast along partitions via rank-1 outer product
                ps_r = spool.tile([HD, SB], F32, name="psr", tag="psr", bufs=1)
                nc.tensor.matmul(
                    ps_r[:], lhsT=ones_row[:], rhs=recip[:],
                    start=True, stop=True)
                rb = opool.tile([HD, SB], F32, name="rb", tag="rb")
                nc.scalar.copy(rb[:], ps_r[:])
                ynorm = ynpool.tile([HD, SB], BF16, name="ynorm", tag="ynorm")
                nc.vector.tensor_tensor(
                    ynorm[:], ps_y[:], rb[:], mybir.AluOpType.mult)
                ynorms.append(ynorm)
            return ynorms

        def wo_block(qb, ynorms, rep=0):
            # row-parallel wo straight out of SBUF: partial out^T over this
            # core's 4 head-dim chunks, then ReduceScatter(add) lands each
            # core's e-shard DIRECTLY in the output parameter -- no device
            # work ever waits on a collective.
            # two half-panels of 16 e-tile chains each: half a's drain DMA
            # and ReduceScatter overlap half b's chains, halving the
            # exposed tail after the final q-block
            # ReduceScatter hands rank r rows [r*256,(r+1)*256) of each
            # half-buffer, so half `hf` must hold, in rank order, the et
            # chunks {4r+2*hf, 4r+2*hf+1} (each core's hf-th half-shard)
            panel = rspool.tile([128, NDC * SB], BF16, name="rsp", tag="rsp")
            for half in range(2):
                for idx in range(NDC // 2):
                    r, k = idx // 2, idx % 2
                    et = 4 * r + 2 * half + k
                    pso = qkvps.tile([128, SB], F32, name="ps", tag="ps")
                    for h in range(QH):
                        nc.tensor.matmul(
                            pso[:],
                            lhsT=wo_big[:, h * D + et * 128:h * D + (et + 1) * 128],
                            rhs=ynorms[h][:],
                            start=(h == 0), stop=(h == QH - 1))
                    nc.vector.tensor_copy(
                        panel[:, (half * 16 + idx) * SB:
                              (half * 16 + idx + 1) * SB], pso[:])
                nc.sync.dma_start(
                    out=rsin[qb][half][:].rearrange("(i p) c -> p i c", p=128),
                    in_=panel[:, half * 16 * SB:(half + 1) * 16 * SB]
                        .rearrange("p (i c) -> p i c", i=NDC // 2))
                if os.environ.get("KERNEL_SIM_NO_COLLECTIVES"):
                    # TimelineSim path: skip the collective (single-core sim
                    # can't model it); keep an equivalently-sized output DMA
                    with tc.tile_wait_until(0.46 * rep + 0.12 + 0.11 * qb
                                            + 0.02 * half):
                        nc.scalar.dma_start(
                            out=out[qb][half * (ESH // 2):(half + 1) * (ESH // 2), :],
                            in_=rsin[qb][half][0:ESH // 2, :])
                    continue
                nc.gpsimd.collective_compute(
                    "ReduceScatter",
                    mybir.AluOpType.add,
                    replica_groups=[list(range(NCORES))],
                    ins=[rsin[qb][half][:].opt()],
                    outs=[rsout[qb][half][:].opt()],
                )
                # tiny DRAM->DRAM hop into the output param (the verifier
                # rejects collectives targeting ExternalOutput directly);
                # wait-hinted to stay out of compute queues
                with tc.tile_wait_until(0.46 * rep + 0.12 + 0.11 * qb
                                        + 0.02 * half):
                    nc.scalar.dma_start(
                        out=out[qb][half * (ESH // 2):(half + 1) * (ESH // 2), :],
                        in_=rsout[qb][half][:])

        # ---- emission schedule ---------------------------------------
        # Everything is local until the trailing ReduceScatter per q-block;
        # reps>1 repeats the whole body (steady-state timing harness).
        for rep in range(reps):
            if rep == 0:
                xh = first_x
            else:
                rsin, rsout = make_rs_tiles(rep)
                xh = [load_x_half(0, 0), load_x_half(0, 1)]
            if causal:
                for sb in range(NSB):
                    qkv_block(sb, xh)
                    if sb + 1 < NSB:
                        nxt0 = load_x_half(sb + 1, 0)
                        nxt1 = load_x_half(sb + 1, 1)
                    yn = attention_block(sb)
                    wo_block(sb, yn, rep)
                    if sb + 1 < NSB:
                        xh = [nxt0, nxt1]
            else:
                for sb in range(NSB):
                    qkv_block(sb, xh)
                    if sb + 1 < NSB:
                        nxt0 = load_x_half(sb + 1, 0)
                        nxt1 = load_x_half(sb + 1, 1)
                        xh = [nxt0, nxt1]
                for qb in range(NSB):
                    yn = attention_block(qb)
                    wo_block(qb, yn, rep)

    nc.finalize()
    return nc


def _build_program_general() -> bass.Bass:
    """Fallback for arbitrary (non-causal, non-zero) masks: the original
    unchunked pipeline with the mask streamed and added before exp."""
    nc = bacc.Bacc(target_bir_lowering=False, trn_type="TRN2")

    xT = nc.declare_dram_parameter("xT", [D, S], BF16, isOutput=False)
    wqkT = nc.declare_dram_parameter("wqkT", [D, EQK], BF16, isOutput=False)
    wvT = nc.declare_dram_parameter("wvT", [D, HD], BF16, isOutput=False)
    woT = nc.declare_dram_parameter("woT", [D, ESH], BF16, isOutput=False)
    cosT = nc.declare_dram_parameter("cosT", [HD, S], BF16, isOutput=False)
    sinT = nc.declare_dram_parameter("sinT", [HD, S], BF16, isOutput=False)
    maskT = nc.declare_dram_parameter("maskT", [S, S], F32, isOutput=False)
    out = nc.declare_dram_parameter("out", [S, ESH], F32, isOutput=True)

    with tile.TileContext(nc) as tc, ExitStack() as ctx:
        persist = ctx.enter_context(tc.tile_pool(name="persist", bufs=1))
        dram = ctx.enter_context(tc.tile_pool(name="dram", bufs=1, space="DRAM"))

        ag_in = dram.tile([QH * HD, S], BF16, name="ag_in", tag="ag_in")
        ag_out = dram.tile([D, S], BF16, name="ag_out", tag="ag_out",
                           addr_space="Shared")

        wqk_sb = []
        for dc in range(NDC):
            t = persist.tile([128, EQK], BF16, name=f"wqk{dc}", tag=f"wqk{dc}")
            nc.sync.dma_start(out=t[:], in_=wqkT[dc * 128:(dc + 1) * 128, :])
            wqk_sb.append(t)
        wv_sb = []
        for dc in range(NDC):
            t = persist.tile([128, HD], BF16, name=f"wv{dc}", tag=f"wv{dc}")
            nc.sync.dma_start(out=t[:], in_=wvT[dc * 128:(dc + 1) * 128, :])
            wv_sb.append(t)
        cos_sb = persist.tile([HD, S], BF16, name="cos", tag="cos")
        nc.sync.dma_start(out=cos_sb[:], in_=cosT[:, :])
        sin_sb = persist.tile([HD, S], BF16, name="sin", tag="sin")
        nc.sync.dma_start(out=sin_sb[:], in_=sinT[:, :])
        ones_sb = persist.tile([128, 1], BF16, name="ones", tag="ones")
        nc.vector.memset(ones_sb[:], 1.0)
        ones_row = persist.tile([1, 128], F32, name="ones_row", tag="ones_row")
        nc.vector.memset(ones_row[:], 1.0)
        ones_f32 = persist.tile([128, 1], F32, name="ones_f32", tag="ones_f32")
        nc.vector.memset(ones_f32[:], 1.0)

        qkT_sb = [persist.tile([HD, S], BF16, name=f"qkT{e}", tag=f"qkT{e}") for e in range(QH + 1)]
        ropT_sb = [persist.tile([HD, S], BF16, name=f"ropT{e}", tag=f"ropT{e}") for e in range(QH + 1)]
        v_sb = persist.tile([128, S], BF16, name="v", tag="v")

        with tc.tile_pool(name="xT", bufs=2 * NDC + 4) as xpool, \
             tc.tile_pool(name="qkvps", bufs=2, space="PSUM") as qkvps, \
             tc.tile_pool(name="ropetmp", bufs=4) as rtmp:
            for sb in range(NSB):
                xts = []
                for dc in range(NDC):
                    t = xpool.tile([128, SB], BF16, name="xt", tag="xt")
                    nc.sync.dma_start(
                        out=t[:], in_=xT[dc * 128:(dc + 1) * 128, sb * SB:(sb + 1) * SB])
                    xts.append(t)
                for et in range(QH + 1):
                    ps = qkvps.tile([128, SB], F32, name="ps", tag="ps")
                    for dc in range(NDC):
                        nc.tensor.matmul(
                            ps[:],
                            lhsT=wqk_sb[dc][:, et * 128:(et + 1) * 128],
                            rhs=xts[dc][:],
                            start=(dc == 0), stop=(dc == NDC - 1))
                    nc.scalar.copy(qkT_sb[et][:, sb * SB:(sb + 1) * SB], ps[:])
                for st in range(SB // 128):
                    ps = qkvps.tile([128, HD], F32, name="psv", tag="psv")
                    for dc in range(NDC):
                        nc.tensor.matmul(
                            ps[:],
                            lhsT=xts[dc][:, st * 128:(st + 1) * 128],
                            rhs=wv_sb[dc][:],
                            start=(dc == 0), stop=(dc == NDC - 1))
                    s0 = sb * SB + st * 128
                    nc.scalar.copy(v_sb[:, s0:s0 + 128], ps[:])

            H2 = HD // 2
            for e in range(QH + 1):
                for sb in range(NSB):
                    sl = slice(sb * SB, (sb + 1) * SB)
                    src = qkT_sb[e]
                    rot = rtmp.tile([128, SB], BF16, name="rot", tag="rot")
                    nc.vector.tensor_copy(rot[0:H2, :], src[H2:HD, sl])
                    nc.vector.tensor_copy(rot[H2:HD, :], src[0:H2, sl])
                    t1 = rtmp.tile([128, SB], BF16, name="t1", tag="t1")
                    nc.vector.tensor_tensor(
                        t1[:], src[:, sl], cos_sb[:, sl], mybir.AluOpType.mult)
                    t2 = rtmp.tile([128, SB], BF16, name="t2", tag="t2")
                    nc.vector.tensor_tensor(
                        t2[:], rot[:], sin_sb[:, sl], mybir.AluOpType.mult)
                    nc.vector.tensor_tensor(
                        ropT_sb[e][:, sl], t1[:], t2[:], mybir.AluOpType.add)

        kT = ropT_sb[QH]
        with ExitStack() as actx:
            mpool = actx.enter_context(tc.tile_pool(name="mask", bufs=NKT + 2))
            spool = actx.enter_context(tc.tile_pool(name="sc_ps", bufs=3, space="PSUM"))
            ypool = actx.enter_context(tc.tile_pool(name="y_ps", bufs=2, space="PSUM"))
            dpool = actx.enter_context(tc.tile_pool(name="d_ps", bufs=2, space="PSUM"))
            epool = actx.enter_context(tc.tile_pool(name="exp", bufs=6))
            opool = actx.enter_context(tc.tile_pool(name="attout", bufs=4))

            for qb in range(NSB):
                qsl = slice(qb * SB, (qb + 1) * SB)
                klim = NKT
                mtiles = []
                for kt in range(klim):
                    mt = mpool.tile([128, SB], F32, name="mt", tag="mt")
                    nc.sync.dma_start(
                        out=mt[:],
                        in_=maskT[kt * 128:(kt + 1) * 128, qsl])
                    mtiles.append(mt)
                for h in range(QH):
                    ps_y = ypool.tile([HD, SB], F32, name="psy", tag="psy")
                    ps_d = dpool.tile([1, SB], F32, name="psd", tag="psd")
                    dsum = opool.tile([128, SB], F32, name="dsum", tag="dsum")
                    for kt in range(klim):
                        ps_s = spool.tile([128, SB], F32, name="pss", tag="pss")
                        nc.tensor.matmul(
                            ps_s[:],
                            lhsT=kT[:, kt * 128:(kt + 1) * 128],
                            rhs=ropT_sb[h][:, qsl],
                            start=True, stop=True)
                        et = epool.tile([128, SB], BF16, name="et", tag="et")
                        nc.vector.tensor_tensor(
                            ps_s[:], ps_s[:], mtiles[kt][:],
                            mybir.AluOpType.add)
                        nc.scalar.activation(
                            et[:], ps_s[:],
                            mybir.ActivationFunctionType.Exp, scale=SCALE)
                        nc.tensor.matmul(
                            ps_y[:],
                            lhsT=v_sb[:, kt * 128:(kt + 1) * 128],
                            rhs=et[:],
                            start=(kt == 0), stop=(kt == klim - 1))
                        if kt == 0:
                            nc.vector.tensor_copy(dsum[:], et[:])
                        else:
                            nc.vector.tensor_tensor(
                                dsum[:], dsum[:], et[:], mybir.AluOpType.add)
                    nc.tensor.matmul(
                        ps_d[:], lhsT=ones_f32[:], rhs=dsum[:],
                        start=True, stop=True)
                    recip = opool.tile([1, SB], F32, name="recip", tag="recip")
                    nc.vector.reciprocal(recip[:], ps_d[:])
                    ps_r = dpool.tile([HD, SB], F32, name="psr", tag="psr", bufs=1)
                    nc.tensor.matmul(
                        ps_r[:], lhsT=ones_row[:], rhs=recip[:],
                        start=True, stop=True)
                    rb = opool.tile([HD, SB], F32, name="rb", tag="rb")
                    nc.scalar.copy(rb[:], ps_r[:])
                    ynorm = opool.tile([HD, SB], BF16, name="ynorm", tag="ynorm")
                    nc.vector.tensor_tensor(
                        ynorm[:], ps_y[:], rb[:], mybir.AluOpType.mult)
                    nc.sync.dma_start(
                        out=ag_in[h * HD:(h + 1) * HD, qsl], in_=ynorm[:])

        nc.gpsimd.collective_compute(
            "AllGather",
            mybir.AluOpType.bypass,
            replica_groups=[list(range(NCORES))],
            ins=[ag_in[:].opt()],
            outs=[ag_out[:].opt()],
        )

        with tc.tile_pool(name="wo", bufs=1) as wpool, \
             tc.tile_pool(name="yt", bufs=NDC + 8) as ytpool, \
             tc.tile_pool(name="ops", bufs=2, space="PSUM") as opsp, \
             tc.tile_pool(name="osb", bufs=4) as osbp:
            wo_sb = []
            for dc in range(NDC):
                t = wpool.tile([128, ESH], BF16, name=f"wo{dc}", tag=f"wo{dc}")
                nc.sync.dma_start(out=t[:], in_=woT[dc * 128:(dc + 1) * 128, :])
                wo_sb.append(t)
            for sg in range(NSB):
                yts = []
                for dc in range(NDC):
                    t = ytpool.tile([128, SB], BF16, name="yt", tag="yt")
                    nc.sync.dma_start(
                        out=t[:],
                        in_=ag_out[dc * 128:(dc + 1) * 128, sg * SB:(sg + 1) * SB])
                    yts.append(t)
                for stl in range(SB // 128):
                    ps = opsp.tile([128, ESH], F32, name="ps", tag="ps")
                    for dc in range(NDC):
                        nc.tensor.matmul(
                            ps[:],
                            lhsT=yts[dc][:, stl * 128:(stl + 1) * 128],
                            rhs=wo_sb[dc][:],
                            start=(dc == 0), stop=(dc == NDC - 1))
                    ot = osbp.tile([128, ESH], F32, name="ot", tag="ot")
                    nc.scalar.copy(ot[:], ps[:])
                    st = sg * (SB // 128) + stl
                    nc.sync.dma_start(
                        out=out[st * 128:(st + 1) * 128, :], in_=ot[:])

    nc.finalize()
    return nc


_PROG_CACHE = {}


def _mask_mode_and_aux(mask):
    m = np.asarray(mask).reshape(S, S)
    if not m.any():
        return "zeros", None
    tril = np.tril(np.ones((S, S), dtype=bool))
    if (m[tril] == 0.0).all() and (m[~tril] == NEG).all():
        return "causal", None
    return "general", np.ascontiguousarray(m.T / SCALE).astype(np.float32)


def _prepare(x, mask, wqkv, wo):
    x = np.asarray(x, dtype=np.float32)
    wqkv = np.asarray(wqkv, dtype=np.float32)
    wo = np.asarray(wo, dtype=np.float32)

    mode, maskT = _mask_mode_and_aux(mask)

    xT = _bf16(x.reshape(S, D).T)                       # [D, S]
    inv = 1.0 / (10000.0 ** (np.arange(0, HD, 2, dtype=np.float32) / HD))
    t = np.arange(S, dtype=np.float32)
    freqs = np.outer(t, inv)                            # [S, HD/2]
    emb = np.concatenate([freqs, freqs], axis=-1)       # [S, HD]
    cosT = _bf16(np.cos(emb).T)                         # [HD, S]
    sinT_np = np.sin(emb).T.copy()                      # [HD, S]
    sinT_np[:HD // 2] *= -1.0                           # bake rotate_half sign
    sinT = _bf16(sinT_np)

    if mode == "causal":
        # additive pattern for diagonal tile j (k0 = qb*512 + j*128):
        # allow when q >= k, i.e. qq >= j*128 + kk  (qq, kk within tile)
        kk = np.arange(128)[:, None]
        qq = np.arange(SB)[None, :]
        pats = []
        for j in range(4):
            allow = qq >= (j * 128 + kk)
            pats.append(np.where(allow, 0.0, NEG / SCALE).astype(np.float32))
        diagp = _bf16(np.concatenate(pats, axis=1))     # [128, 2048] bf16

    in_maps = []
    for r in range(NCORES):
        q_rows = wqkv[r * QH * HD:(r + 1) * QH * HD]            # [512, D]
        k_rows = wqkv[NH * HD + r * HD: NH * HD + (r + 1) * HD]  # [128, D]
        v_rows = wqkv[(NH + NKV) * HD + r * HD:(NH + NKV) * HD + (r + 1) * HD]
        im = {
            "xT": xT,
            "wqkT": _bf16(np.concatenate([q_rows, k_rows], axis=0).T),  # [D, 640]
            "wvT": _bf16(v_rows.T),                                     # [D, 128]
            "cosT": cosT,
            "sinT": sinT,
        }
        if mode == "general":
            # column-parallel wo (post-AllGather): [D, 512] e-shard
            im["woT"] = _bf16(wo[r * ESH:(r + 1) * ESH, :].T)
        else:
            # row-parallel wo (pre-ReduceScatter): this core's 512
            # head-dims x all 4096 output cols -> [512, D]
            im["woT"] = _bf16(wo[:, r * ESH:(r + 1) * ESH].T)
        if mode == "general":
            im["maskT"] = maskT
        elif mode == "causal":
            im["diagp"] = diagp
        in_maps.append(im)
    return mode, in_maps


def kernel(x, mask, wqkv, wo):
    global LAST_RESULT
    mode, in_maps = _prepare(x, mask, wqkv, wo)

    if mode not in _PROG_CACHE:
        _PROG_CACHE[mode] = _build_program(mode)
    nc = _PROG_CACHE[mode]

    res = bass_utils.run_bass_kernel_spmd(
        nc, in_maps, core_ids=list(range(NCORES)),
        trace=bool(os.environ.get("BASS_TRACE")),
    )
    LAST_RESULT = res

    if mode == "general":
        shards = [np.asarray(res.results[r]["out"], dtype=np.float32)
                  for r in range(NCORES)]
        full = np.concatenate(shards, axis=1)           # [S, D]
    else:
        # out is [NSB, ESH, SB] bf16 = per-q-block out^T shards
        shards = []
        for r in range(NCORES):
            o = np.asarray(res.results[r]["out"], dtype=np.float32)
            shards.append(np.concatenate(list(o), axis=1).T)   # [S, ESH]
        full = np.concatenate(shards, axis=1)           # [S, D]
    return full.reshape(B, S, D)



# revision 32
# speedup vs baseline: 3.9523x; 3.9523x over previous
"""LlamaAttention (B=1,S=2048,D=4096,NH=32,NKV=8,HD=128) on 8 TRN2 NeuronCores.

Sharding: tensor-parallel over heads (4 Q heads + 1 KV head per core).
Everything on-device runs in a transposed [feature, seq] layout so no PE
transposes are needed anywhere:
  - host ships x^T, wqkv^T-shard, wo^T-shard, cos^T/sin^T as bf16
  - QKV projection produces Q^T/K^T directly; V is produced in natural
    [seq, hd] layout (it is the AV matmul's stationary operand)
  - scores_T[k,q] = (K^T)^T . Q^T per 128x512 tile; exp on ACT engine
  - softmax denominator: exp tiles pair-summed on GpSimd, then a
    ones-vector matmul chain on PE reduces over partitions
  - y^T accumulated in PSUM, normalized with a partition-broadcast
    reciprocal (rank-1 ones outer product on PE)
  - wo is ROW-PARALLEL (Megatron style): each core contracts its own 4
    heads' y^T straight out of SBUF into a partial out^T[4096e, 512s]
    per q-block -- no gather, no DRAM round trip of y
  - a per-q-block ReduceScatter(add) sums the partials and lands each
    core's 512-col e-shard; no compute ever waits on a collective (the
    only post-collective op is a tiny DRAM->DRAM hop into the output)
  - host transposes/concatenates the 8 transposed column shards
All bulk HBM traffic uses batched 3D-access-pattern DMAs (one descriptor
per multi-tile panel) to keep the DMA-trigger sequencers off the
critical path, spread across the SP/ACT/DVE trigger queues.
Mask handling is chosen host-side: causal fast path (skip upper-tri
k-tiles, additive diagonal patterns), all-zeros path (no mask at all), or
general path (stream mask^T/scale tiles and add before exp).
"""

import os
import sys
from contextlib import ExitStack

sys.path.insert(0, "/opt/trn_rl_repo")

import ml_dtypes
import numpy as np

import concourse.bass as bass
import concourse.mybir as mybir
import concourse.tile as tile
from concourse import bacc, bass_utils

F32 = mybir.dt.float32
BF16 = mybir.dt.bfloat16

B, S, D = 1, 2048, 4096
NH, NKV, HD = 32, 8, 128
NCORES = 8
QH = NH // NCORES            # 4 Q heads per core
EQK = QH * HD + HD           # 640 cols of wqkT per core (4 Q heads + 1 K head)
ESH = D // NCORES            # 512 output cols per core
SCALE = 1.0 / float(np.sqrt(HD))
NEG = -1e9

SB = 512                     # seq block (matmul free dim)
NSB = S // SB                # 4
NKT = S // 128               # 16 k tiles
NDC = D // 128               # 32 contraction chunks

# scheduling hints (ms) for the post-ReduceScatter DRAM->DRAM output hop
HINT_REP = 0.41
HINT_BASE = 0.105
HINT_QB = 0.095
HINT_HALF = 0.018

LAST_RESULT = None           # BassKernelResults of the most recent run


def _bf16(a):
    return np.ascontiguousarray(a).astype(ml_dtypes.bfloat16)


def _build_program(mask_mode: str, reps: int = 1) -> bass.Bass:
    if mask_mode == "general":
        return _build_program_general()

    causal = mask_mode == "causal"
    nc = bacc.Bacc(target_bir_lowering=False, trn_type="TRN2")

    xT = nc.declare_dram_parameter("xT", [D, S], BF16, isOutput=False)
    wqkT = nc.declare_dram_parameter("wqkT", [D, EQK], BF16, isOutput=False)
    wvT = nc.declare_dram_parameter("wvT", [D, HD], BF16, isOutput=False)
    # row-parallel wo: this core's 512 head-dims x all 4096 output cols
    woT = nc.declare_dram_parameter("woT", [ESH, D], BF16, isOutput=False)
    cosT = nc.declare_dram_parameter("cosT", [HD, S], BF16, isOutput=False)
    sinT = nc.declare_dram_parameter("sinT", [HD, S], BF16, isOutput=False)
    if causal:
        diagp = nc.declare_dram_parameter("diagp", [128, 4 * SB], BF16, isOutput=False)
    # out^T shard per q-block, written DIRECTLY by the ReduceScatter
    out = nc.declare_dram_parameter("out", [NSB, ESH, SB], BF16, isOutput=True)

    with tile.TileContext(nc) as tc, ExitStack() as ctx:
        persist = ctx.enter_context(tc.tile_pool(name="persist", bufs=1))
        dram = ctx.enter_context(tc.tile_pool(name="dram", bufs=1, space="DRAM"))

        def make_rs_tiles(rep):
            sfx = f"_r{rep}" if rep else ""
            rsi = [[dram.tile([D // 2, SB], BF16, name=f"rsin{q}{hf}{sfx}",
                              tag=f"rsin{q}{hf}{sfx}") for hf in "ab"]
                   for q in range(NSB)]
            rso = [[dram.tile([ESH // 2, SB], BF16, name=f"rsout{q}{hf}{sfx}",
                              tag=f"rsout{q}{hf}{sfx}") for hf in "ab"]
                   for q in range(NSB)]
            return rsi, rso

        rsin, rsout = make_rs_tiles(0)

        # ---- resident weights / tables -------------------------------
        # the boot stream interleaves wqk / x / wv panels in dc
        # (contraction-chunk) order so the dc-outer boot block below can
        # start computing ~2.5us in and stream behind the DMA arrivals
        wqk_big = persist.tile([128, NDC * EQK], BF16, name="wqk", tag="wqk")
        wv_big = persist.tile([128, NDC * HD], BF16, name="wv", tag="wv")
        xpool = ctx.enter_context(tc.tile_pool(name="x", bufs=2))

        def load_x_half(sb, half, tile_=None):
            t = tile_ if tile_ is not None else xpool.tile(
                [128, 16 * SB], BF16, name="xh", tag="xh")
            nc.sync.dma_start(
                out=t[:].rearrange("p (i c) -> p i c", i=16),
                in_=xT[half * 2048:(half + 1) * 2048,
                       sb * SB:(sb + 1) * SB]
                    .rearrange("(i p) c -> p i c", p=128))
            return t

        def load_x_piece(t, half, dc0, w):
            # boot only (sb=0); dc0 relative to the half
            nc.sync.dma_start(
                out=t[:, dc0 * SB:(dc0 + w) * SB]
                    .rearrange("p (i c) -> p i c", i=w),
                in_=xT[half * 2048 + dc0 * 128:
                       half * 2048 + (dc0 + w) * 128, 0:SB]
                    .rearrange("(i p) c -> p i c", p=128))

        def load_wqk(g, n):
            nc.sync.dma_start(
                out=wqk_big[:, g * EQK:(g + n) * EQK]
                    .rearrange("p (i e) -> p i e", i=n),
                in_=wqkT[g * 128:(g + n) * 128, :]
                    .rearrange("(i p) e -> p i e", p=128))

        def load_wv(g, n):
            nc.sync.dma_start(
                out=wv_big[:, g * HD:(g + n) * HD]
                    .rearrange("p (i e) -> p i e", i=n),
                in_=wvT[g * 128:(g + n) * 128, :]
                    .rearrange("(i p) e -> p i e", p=128))

        x00 = xpool.tile([128, 16 * SB], BF16, name="xh", tag="xh")
        x01 = xpool.tile([128, 16 * SB], BF16, name="xh", tag="xh")
        for g in range(0, NDC, 4):
            load_wqk(g, 4)
            if g < 16:
                load_x_piece(x00, 0, g, 4)
            else:
                load_x_piece(x01, 1, g - 16, 4)
            load_wv(g, 4)
        first_x = [x00, x01]

        # non-boot-critical tables queue behind the boot stream
        cos_sb = persist.tile([HD, S], BF16, name="cos", tag="cos")
        nc.sync.dma_start(out=cos_sb[:], in_=cosT[:, :])
        sin_sb = persist.tile([HD, S], BF16, name="sin", tag="sin")
        nc.sync.dma_start(out=sin_sb[:], in_=sinT[:, :])
        if causal:
            diag_sb = persist.tile([128, 4 * SB], BF16, name="diag", tag="diag")
            nc.sync.dma_start(out=diag_sb[:], in_=diagp[:, :])
        # wo panel: 4 local-head d-chunks x 4096 e cols
        wo_big = persist.tile([128, QH * D], BF16, name="wo", tag="wo")
        nc.sync.dma_start(
            out=wo_big[:].rearrange("p (i e) -> p i e", i=QH),
            in_=woT[:].rearrange("(i p) e -> p i e", p=128))

        ones_bf = persist.tile([128, 1], BF16, name="ones_bf", tag="ones_bf")
        nc.vector.memset(ones_bf[:], 1.0)
        ones_row = persist.tile([1, 128], BF16, name="ones_row", tag="ones_row")
        nc.vector.memset(ones_row[:], 1.0)

        # persistent activations: Q^T per head + K^T (RoPE'd in place), V
        qkT = [persist.tile([HD, S], BF16, name=f"qkT{e}", tag=f"qkT{e}")
               for e in range(QH + 1)]
        v_sb = persist.tile([128, S], BF16, name="v", tag="v")

        qkvps = ctx.enter_context(tc.tile_pool(name="qkvps", bufs=2, space="PSUM"))
        spool = ctx.enter_context(tc.tile_pool(name="sc_ps", bufs=3, space="PSUM"))
        ypool = ctx.enter_context(tc.tile_pool(name="y_ps", bufs=2, space="PSUM"))
        dpool = ctx.enter_context(tc.tile_pool(name="d_ps", bufs=1, space="PSUM"))
        rtmp = ctx.enter_context(tc.tile_pool(name="rtmp", bufs=1))
        epool = ctx.enter_context(tc.tile_pool(name="exp", bufs=5))
        ppool = ctx.enter_context(tc.tile_pool(name="pair", bufs=2))
        opool = ctx.enter_context(tc.tile_pool(name="attout", bufs=2))
        # 8 = two full head-sets: wo(qb) half-b chains keep reading
        # ynorms(qb) deep into attention(qb+1) while its tails write new ones
        ynpool = ctx.enter_context(tc.tile_pool(name="ynorm", bufs=8))
        rspool = ctx.enter_context(tc.tile_pool(name="rsp", bufs=1))

        H2 = HD // 2

        def rope_slice(et, sb):
            # RoPE in place on a fresh [HD, SB] slice (DVE)
            sl = slice(sb * SB, (sb + 1) * SB)
            src = qkT[et]
            rot = rtmp.tile([128, SB], BF16, name="rot", tag="rot")
            nc.vector.tensor_copy(rot[0:H2, :], src[H2:HD, sl])
            nc.vector.tensor_copy(rot[H2:HD, :], src[0:H2, sl])
            t1 = rtmp.tile([128, SB], BF16, name="t1", tag="t1")
            nc.vector.tensor_tensor(
                t1[:], src[:, sl], cos_sb[:, sl], mybir.AluOpType.mult)
            t2 = rtmp.tile([128, SB], BF16, name="t2", tag="t2")
            nc.vector.tensor_tensor(
                t2[:], rot[:], sin_sb[:, sl], mybir.AluOpType.mult)
            nc.vector.tensor_tensor(
                src[:, sl], t1[:], t2[:], mybir.AluOpType.add)

        def wqk_sl(dc, et):
            return wqk_big[:, dc * EQK + et * 128:dc * EQK + (et + 1) * 128]

        def xh_sl(xh, dc):
            return xh[dc // 16][:, (dc % 16) * SB:(dc % 16 + 1) * SB]

        def qkv_boot(xh):
            # sb=0 of rep 0: the 5 K/Q chains run dc-outer across 5 psum
            # banks so the PE streams right behind the dc-ordered DMA
            # arrivals. V runs st-outer afterwards: its 4 column-slice
            # groups share ONE bank, and start_tensor_calc zeroes the whole
            # 2KB zero region, so those groups must stay sequential.
            ps_k = qkvps.tile([128, SB], F32, name="ps", tag="ps")
            ps_q = [qkvps.tile([128, SB], F32, name="ps", tag="ps"),
                    spool.tile([128, SB], F32, name="pss", tag="pss"),
                    spool.tile([128, SB], F32, name="pss", tag="pss"),
                    spool.tile([128, SB], F32, name="pss", tag="pss")]
            for dc in range(NDC):
                st, sp = (dc == 0), (dc == NDC - 1)
                nc.tensor.matmul(ps_k[:], lhsT=wqk_sl(dc, QH),
                                 rhs=xh_sl(xh, dc), start=st, stop=sp)
                for h in range(QH):
                    nc.tensor.matmul(ps_q[h][:], lhsT=wqk_sl(dc, h),
                                     rhs=xh_sl(xh, dc), start=st, stop=sp)
            # drains + RoPE; K, Q0 first so attention(0) can start early
            nc.scalar.copy(qkT[QH][:, 0:SB], ps_k[:])
            rope_slice(QH, 0)
            nc.scalar.copy(qkT[0][:, 0:SB], ps_q[0][:])
            rope_slice(0, 0)
            qkv_v(0, xh)
            for h in range(1, QH):
                nc.scalar.copy(qkT[h][:, 0:SB], ps_q[h][:])
                rope_slice(h, 0)

        def qkv_chain(et, sb, xh):
            ps = qkvps.tile([128, SB], F32, name="ps", tag="ps")
            for dc in range(NDC):
                nc.tensor.matmul(
                    ps[:], lhsT=wqk_sl(dc, et), rhs=xh_sl(xh, dc),
                    start=(dc == 0), stop=(dc == NDC - 1))
            nc.scalar.copy(qkT[et][:, sb * SB:(sb + 1) * SB], ps[:])
            rope_slice(et, sb)

        def qkv_chain_seg(et, sb, xh):
            # boundary filler: the chain is emitted in segments which the
            # caller interleaves with the previous block's deferred softmax
            # tails. Runs in a scores-ring bank so it doesn't contend with
            # wo chains for the qkvps ring.
            ps = spool.tile([128, SB], F32, name="pss", tag="pss")

            def emit(d0, d1):
                for dc in range(d0, d1):
                    nc.tensor.matmul(
                        ps[:], lhsT=wqk_sl(dc, et), rhs=xh_sl(xh, dc),
                        start=(dc == 0), stop=(dc == NDC - 1))

            def finish():
                nc.scalar.copy(qkT[et][:, sb * SB:(sb + 1) * SB], ps[:])
                rope_slice(et, sb)

            return emit, finish

        def qkv_v(sb, xh):
            # V natural: 4 chains of 32 matmuls (FD=128) into free-dim
            # slices of one shared psum tile, drained with a single copy
            psv = qkvps.tile([128, SB], F32, name="ps", tag="ps")
            for st in range(SB // 128):
                for dc in range(NDC):
                    nc.tensor.matmul(
                        psv[:, st * 128:(st + 1) * 128],
                        lhsT=xh[dc // 16][:, (dc % 16) * SB + st * 128:
                                          (dc % 16) * SB + (st + 1) * 128],
                        rhs=wv_big[:, dc * HD:(dc + 1) * HD],
                        start=(dc == 0), stop=(dc == NDC - 1))
            nc.scalar.copy(v_sb[:, sb * SB:(sb + 1) * SB], psv[:])

        def qkv_block(sb, xh, ets):
            # K chain first so its drain+RoPE overlap the remaining chains
            for et in ets:
                qkv_chain(et, sb, xh)
            qkv_v(sb, xh)

        def attention_block(qb, fill=lambda n=1: None):
            # Software-pipelined emission: 2-deep scores lookahead so the
            # exp (ACT) latency hides under AV matmuls; denominator summed
            # 4-wide on GpSimd (pairs then quads, Pool-only, in-loop); the
            # PE reduction over quads + reciprocal for head h is deferred
            # into head h+1's stream (tail_d, at kt=4) and the broadcast/
            # normalize into head h+2's prologue (tail_r) so the ~2us
            # exp->pair->quad cross-engine latency never stalls the
            # in-order PE queue.
            qsl = slice(qb * SB, (qb + 1) * SB)
            klim = (qb + 1) * (SB // 128) if causal else NKT
            nquad = klim // 4
            td_kt = min(4, klim - 2)
            ynorms = []
            tails_d = {}
            tails_r = {}

            def emit_head(h):
                ets = {}
                prs = {}
                qrs = {}

                def score(kt):
                    ps_s = spool.tile([128, SB], F32, name="pss", tag="pss")
                    nc.tensor.matmul(
                        ps_s[:],
                        lhsT=qkT[QH][:, kt * 128:(kt + 1) * 128],
                        rhs=qkT[h][:, qsl],
                        start=True, stop=True)
                    if causal and kt >= qb * (SB // 128):
                        j = kt - qb * (SB // 128)
                        nc.vector.tensor_tensor(
                            ps_s[:], ps_s[:],
                            diag_sb[:, j * SB:(j + 1) * SB],
                            mybir.AluOpType.add)
                    et = epool.tile([128, SB], BF16, name="et", tag="et")
                    nc.scalar.activation(
                        et[:], ps_s[:],
                        mybir.ActivationFunctionType.Exp, scale=SCALE)
                    ets[kt] = et

                def pair(j):
                    # DVE: ~2x faster than GpSimd for [128,SB] bf16 adds;
                    # quads stay on GpSimd to keep both engines under the
                    # PE/ACT pace
                    pr = ppool.tile([128, SB], BF16, name="pr", tag="pr")
                    nc.vector.tensor_tensor(
                        pr[:], ets[2 * j][:], ets[2 * j + 1][:],
                        mybir.AluOpType.add)
                    prs[j] = pr

                def quad(q):
                    qr = ppool.tile([128, SB], BF16, name="qr", tag="qr",
                                    bufs=4)
                    nc.gpsimd.tensor_tensor(
                        qr[:], prs.pop(2 * q)[:], prs.pop(2 * q + 1)[:],
                        mybir.AluOpType.add)
                    qrs[q] = qr

                score(0)
                score(1)
                if h >= 2:
                    tails_r.pop(h - 2)()
                ps_y = ypool.tile([HD, SB], F32, name="psy", tag="psy")
                done_p, done_q = set(), set()
                for kt in range(klim):
                    if kt in (4, 8, 12) or (klim == 4 and kt == 2):
                        # one filler chain per ~4 k-tiles absorbs the exp
                        # (ACT) pacing deficit so the AV matmuls never wait
                        fill(1)
                    if kt == td_kt and h >= 1:
                        tails_d.pop(h - 1)()
                    nc.tensor.matmul(
                        ps_y[:],
                        lhsT=v_sb[:, kt * 128:(kt + 1) * 128],
                        rhs=ets[kt][:],
                        start=(kt == 0), stop=(kt == klim - 1))
                    if kt + 2 < klim:
                        score(kt + 2)
                    if kt >= 2 and kt % 2 == 0:
                        j = (kt - 2) // 2
                        pair(j)
                        done_p.add(j)
                    if kt >= 5 and (kt - 5) % 4 == 0:
                        q = (kt - 5) // 4
                        quad(q)
                        done_q.add(q)
                # leftover Pool work now (never stalls the PE queue)
                for j in range(klim // 2):
                    if j not in done_p:
                        pair(j)
                for q in range(nquad):
                    if q not in done_q:
                        quad(q)

                def tail_d():
                    ps_d = dpool.tile([1, SB], F32, name="psd", tag="psd")
                    for q in range(nquad):
                        nc.tensor.matmul(
                            ps_d[:], lhsT=ones_bf[:], rhs=qrs[q][:],
                            start=(q == 0), stop=(q == nquad - 1))
                    recip = opool.tile([1, SB], BF16, name="recip",
                                       tag="recip")
                    with nc.allow_low_precision(
                            reason="softmax denom is positive and O(100); "
                                   "bf16 reciprocal feeds a bf16 broadcast "
                                   "anyway"):
                        nc.vector.reciprocal(recip[:], ps_d[:])
                    tails_r[h] = make_tail_r(recip)

                def make_tail_r(recip):
                    def tail_r():
                        # broadcast along partitions via rank-1 outer product
                        ps_r = spool.tile([HD, SB], F32, name="psr",
                                          tag="pss")
                        nc.tensor.matmul(
                            ps_r[:], lhsT=ones_row[:], rhs=recip[:],
                            start=True, stop=True)
                        rb = opool.tile([HD, SB], F32, name="rb", tag="rb",
                                        bufs=1)
                        nc.scalar.copy(rb[:], ps_r[:])
                        ynorm = ynpool.tile([HD, SB], BF16, name="ynorm",
                                            tag="ynorm")
                        nc.vector.tensor_tensor(
                            ynorm[:], ps_y[:], rb[:], mybir.AluOpType.mult)
                        ynorms.append(ynorm)
                    return tail_r

                tails_d[h] = tail_d

            for h in range(QH):
                emit_head(h)
                fill(1)
            # leftovers for the caller to interleave with boundary PE work:
            # tail_r(QH-2), tail_d(QH-1), tail_r(QH-1) -- in this order
            pending = [tails_r.pop(QH - 2)]

            def _d_last():
                tails_d.pop(QH - 1)()

            def _r_last():
                tails_r.pop(QH - 1)()

            pending.append(_d_last)
            pending.append(_r_last)
            return ynorms, pending

        # ---- wo: row-parallel straight out of SBUF --------------------
        # partial out^T over this core's 4 head-dim chunks, then a
        # ReduceScatter(add) per half-panel lands each core's e-shard
        # DIRECTLY in the output parameter -- no device work ever waits on
        # a collective. ReduceScatter hands rank r rows [r*256,(r+1)*256)
        # of each half-buffer, so half `hf` must hold, in rank order, the
        # et chunks {4r+2*hf, 4r+2*hf+1} (each core's hf-th half-shard).
        # Half a is emitted contiguously after its q-block; half b is
        # sliced into 16 single-chain closures used as PE filler inside
        # the NEXT q-block's attention (each chain covers the exp/ACT
        # pacing deficit of ~4 k-tiles).

        def wo_chain(ynorms, panel, half, idx):
            r, k = idx // 2, idx % 2
            et = 4 * r + 2 * half + k
            pso = qkvps.tile([128, SB], F32, name="ps", tag="ps")
            for h in range(QH):
                nc.tensor.matmul(
                    pso[:],
                    lhsT=wo_big[:, h * D + et * 128:h * D + (et + 1) * 128],
                    rhs=ynorms[h][:],
                    start=(h == 0), stop=(h == QH - 1))
            nc.vector.tensor_copy(
                panel[:, idx * SB:(idx + 1) * SB], pso[:])

        def wo_half_send(qb, panel, half, rep):
            hint = (HINT_REP * rep + HINT_BASE + HINT_QB * qb
                    + HINT_HALF * half)
            nc.sync.dma_start(
                out=rsin[qb][half][:].rearrange("(i p) c -> p i c", p=128),
                in_=panel[:].rearrange("p (i c) -> p i c", i=NDC // 2))
            if os.environ.get("KERNEL_SIM_NO_COLLECTIVES"):
                # TimelineSim path: skip the collective (single-core sim
                # can't model it); keep an equivalently-sized output DMA
                with tc.tile_wait_until(hint):
                    nc.scalar.dma_start(
                        out=out[qb][half * (ESH // 2):(half + 1) * (ESH // 2), :],
                        in_=rsin[qb][half][0:ESH // 2, :])
                return
            nc.gpsimd.collective_compute(
                "ReduceScatter",
                mybir.AluOpType.add,
                replica_groups=[list(range(NCORES))],
                ins=[rsin[qb][half][:].opt()],
                outs=[rsout[qb][half][:].opt()],
            )
            # tiny DRAM->DRAM hop into the output param (the verifier
            # rejects collectives targeting ExternalOutput directly);
            # wait-hinted to stay out of compute queues
            with tc.tile_wait_until(hint):
                nc.scalar.dma_start(
                    out=out[qb][half * (ESH // 2):(half + 1) * (ESH // 2), :],
                    in_=rsout[qb][half][:])

        def wo_half(qb, ynorms, rep, half):
            panel = rspool.tile([128, (NDC // 2) * SB], BF16,
                                name=f"rsp{half}", tag=f"rsp{half}")
            for idx in range(NDC // 2):
                wo_chain(ynorms, panel, half, idx)
            wo_half_send(qb, panel, half, rep)

        def wo_half_fillers(qb, ynorms, rep, half=1):
            state = {}

            def mk(idx):
                def f():
                    if "panel" not in state:
                        state["panel"] = rspool.tile(
                            [128, (NDC // 2) * SB], BF16,
                            name=f"rsp{half}", tag=f"rsp{half}")
                    wo_chain(ynorms, state["panel"], half, idx)
                    if idx == NDC // 2 - 1:
                        wo_half_send(qb, state["panel"], half, rep)
                return f

            return [mk(i) for i in range(NDC // 2)]

        # ---- emission schedule ---------------------------------------
        # Everything is local until the trailing ReduceScatter per q-block;
        # reps>1 repeats the whole body (steady-state timing harness).
        fillq = []

        def fill(n=1):
            for _ in range(min(n, len(fillq))):
                fillq.pop(0)()

        def qkv_chain_half_closures(et, sb, xh):
            # a chain split into two filler closures; adjacent in the fill
            # queue so at most ~1 chain's psum tile is in flight at a time
            state = {}

            def first():
                state["ps"] = qkvps.tile([128, SB], F32, name="ps", tag="ps")
                for dc in range(NDC // 2):
                    nc.tensor.matmul(
                        state["ps"][:], lhsT=wqk_sl(dc, et),
                        rhs=xh_sl(xh, dc),
                        start=(dc == 0), stop=False)

            def second():
                for dc in range(NDC // 2, NDC):
                    nc.tensor.matmul(
                        state["ps"][:], lhsT=wqk_sl(dc, et),
                        rhs=xh_sl(xh, dc),
                        start=False, stop=(dc == NDC - 1))
                nc.scalar.copy(qkT[et][:, sb * SB:(sb + 1) * SB],
                               state["ps"][:])
                rope_slice(et, sb)

            return [first, second]

        def boundary(pending, chain_et=None, sbn=None, xh_next=None):
            # flush the last heads' deferred softmax tails with PE work
            # (leftover fillers + the next block's K/Q3 chain in segments)
            # between them so the cross-engine latency chains stay hidden
            if chain_et is not None:
                emit, fin = qkv_chain_seg(chain_et, sbn, xh_next)
                for (d0, d1), p in zip(
                        [(0, 8), (8, 12), (12, 16), (16, NDC)],
                        pending + [None]):
                    fill(2)
                    emit(d0, d1)
                    if p is not None:
                        p()
                fin()
                fill(len(fillq))
            else:
                fill(len(fillq))
                for p in pending:
                    p()

        pre_x = None
        for rep in range(reps):
            if rep == 0:
                xh = first_x
            else:
                rsin, rsout = make_rs_tiles(rep)
                xh = pre_x if pre_x else [load_x_half(0, 0),
                                          load_x_half(0, 1)]
            if causal:
                # --- sb 0 ---
                if rep == 0:
                    qkv_boot(xh)
                else:
                    qkv_block(0, xh,
                              [0, 1, 2, 3] if pre_x else [QH, 0, 1, 2, 3])
                pre_x = None
                nxt = [load_x_half(1, 0), load_x_half(1, 1)]
                for et in (QH, 0, 1, 2):
                    fillq.extend(qkv_chain_half_closures(et, 1, nxt))
                yn, pending = attention_block(0, fill)
                boundary(pending, chain_et=3, sbn=1, xh_next=nxt)
                qkv_v(1, nxt)
                wo_half(0, yn, rep, 0)
                prev_yn, xh = yn, nxt
                # --- sb 1..3 ---
                for sb in range(1, NSB):
                    if sb + 1 < NSB:
                        nxt = [load_x_half(sb + 1, 0), load_x_half(sb + 1, 1)]
                    fillq.extend(wo_half_fillers(sb - 1, prev_yn, rep))
                    yn, pending = attention_block(sb, fill)
                    if sb + 1 < NSB:
                        boundary(pending, chain_et=QH, sbn=sb + 1,
                                 xh_next=nxt)
                        qkv_block(sb + 1, nxt, [0, 1, 2, 3])
                        wo_half(sb, yn, rep, 0)
                    else:
                        if rep + 1 < reps:
                            # flush the tails against the NEXT rep's K
                            # chain so the last q-block boundary also
                            # stays gapless in steady state
                            pre_x = [load_x_half(0, 0), load_x_half(0, 1)]
                            boundary(pending, chain_et=QH, sbn=0,
                                     xh_next=pre_x)
                        else:
                            boundary(pending)
                        wo_half(sb, yn, rep, 0)
                        wo_half(sb, yn, rep, 1)
                    prev_yn, xh = yn, nxt
            else:
                for sb in range(NSB):
                    if rep == 0 and sb == 0:
                        qkv_boot(xh)
                    else:
                        qkv_block(sb, xh, [QH, 0, 1, 2, 3])
                    if sb + 1 < NSB:
                        xh = [load_x_half(sb + 1, 0), load_x_half(sb + 1, 1)]
                for qb in range(NSB):
                    yn, pending = attention_block(qb)
                    for p in pending:
                        p()
                    wo_half(qb, yn, rep, 0)
                    wo_half(qb, yn, rep, 1)

    nc.finalize()
    return nc


def _build_program_general() -> bass.Bass:
    """Fallback for arbitrary (non-causal, non-zero) masks: the original
    unchunked pipeline with the mask streamed and added before exp."""
    nc = bacc.Bacc(target_bir_lowering=False, trn_type="TRN2")

    xT = nc.declare_dram_parameter("xT", [D, S], BF16, isOutput=False)
    wqkT = nc.declare_dram_parameter("wqkT", [D, EQK], BF16, isOutput=False)
    wvT = nc.declare_dram_parameter("wvT", [D, HD], BF16, isOutput=False)
    woT = nc.declare_dram_parameter("woT", [D, ESH], BF16, isOutput=False)
    cosT = nc.declare_dram_parameter("cosT", [HD, S], BF16, isOutput=False)
    sinT = nc.declare_dram_parameter("sinT", [HD, S], BF16, isOutput=False)
    maskT = nc.declare_dram_parameter("maskT", [S, S], F32, isOutput=False)
    out = nc.declare_dram_parameter("out", [S, ESH], F32, isOutput=True)

    with tile.TileContext(nc) as tc, ExitStack() as ctx:
        persist = ctx.enter_context(tc.tile_pool(name="persist", bufs=1))
        dram = ctx.enter_context(tc.tile_pool(name="dram", bufs=1, space="DRAM"))

        ag_in = dram.tile([QH * HD, S], BF16, name="ag_in", tag="ag_in")
        ag_out = dram.tile([D, S], BF16, name="ag_out", tag="ag_out",
                           addr_space="Shared")

        wqk_sb = []
        for dc in range(NDC):
            t = persist.tile([128, EQK], BF16, name=f"wqk{dc}", tag=f"wqk{dc}")
            nc.sync.dma_start(out=t[:], in_=wqkT[dc * 128:(dc + 1) * 128, :])
            wqk_sb.append(t)
        wv_sb = []
        for dc in range(NDC):
            t = persist.tile([128, HD], BF16, name=f"wv{dc}", tag=f"wv{dc}")
            nc.sync.dma_start(out=t[:], in_=wvT[dc * 128:(dc + 1) * 128, :])
            wv_sb.append(t)
        cos_sb = persist.tile([HD, S], BF16, name="cos", tag="cos")
        nc.sync.dma_start(out=cos_sb[:], in_=cosT[:, :])
        sin_sb = persist.tile([HD, S], BF16, name="sin", tag="sin")
        nc.sync.dma_start(out=sin_sb[:], in_=sinT[:, :])
        ones_sb = persist.tile([128, 1], BF16, name="ones", tag="ones")
        nc.vector.memset(ones_sb[:], 1.0)
        ones_row = persist.tile([1, 128], F32, name="ones_row", tag="ones_row")
        nc.vector.memset(ones_row[:], 1.0)
        ones_f32 = persist.tile([128, 1], F32, name="ones_f32", tag="ones_f32")
        nc.vector.memset(ones_f32[:], 1.0)

        qkT_sb = [persist.tile([HD, S], BF16, name=f"qkT{e}", tag=f"qkT{e}") for e in range(QH + 1)]
        ropT_sb = [persist.tile([HD, S], BF16, name=f"ropT{e}", tag=f"ropT{e}") for e in range(QH + 1)]
        v_sb = persist.tile([128, S], BF16, name="v", tag="v")

        with tc.tile_pool(name="xT", bufs=2 * NDC + 4) as xpool, \
             tc.tile_pool(name="qkvps", bufs=2, space="PSUM") as qkvps, \
             tc.tile_pool(name="ropetmp", bufs=4) as rtmp:
            for sb in range(NSB):
                xts = []
                for dc in range(NDC):
                    t = xpool.tile([128, SB], BF16, name="xt", tag="xt")
                    nc.sync.dma_start(
                        out=t[:], in_=xT[dc * 128:(dc + 1) * 128, sb * SB:(sb + 1) * SB])
                    xts.append(t)
                for et in range(QH + 1):
                    ps = qkvps.tile([128, SB], F32, name="ps", tag="ps")
                    for dc in range(NDC):
                        nc.tensor.matmul(
                            ps[:],
                            lhsT=wqk_sb[dc][:, et * 128:(et + 1) * 128],
                            rhs=xts[dc][:],
                            start=(dc == 0), stop=(dc == NDC - 1))
                    nc.scalar.copy(qkT_sb[et][:, sb * SB:(sb + 1) * SB], ps[:])
                for st in range(SB // 128):
                    ps = qkvps.tile([128, HD], F32, name="psv", tag="psv")
                    for dc in range(NDC):
                        nc.tensor.matmul(
                            ps[:],
                            lhsT=xts[dc][:, st * 128:(st + 1) * 128],
                            rhs=wv_sb[dc][:],
                            start=(dc == 0), stop=(dc == NDC - 1))
                    s0 = sb * SB + st * 128
                    nc.scalar.copy(v_sb[:, s0:s0 + 128], ps[:])

            H2 = HD // 2
            for e in range(QH + 1):
                for sb in range(NSB):
                    sl = slice(sb * SB, (sb + 1) * SB)
                    src = qkT_sb[e]
                    rot = rtmp.tile([128, SB], BF16, name="rot", tag="rot")
                    nc.vector.tensor_copy(rot[0:H2, :], src[H2:HD, sl])
                    nc.vector.tensor_copy(rot[H2:HD, :], src[0:H2, sl])
                    t1 = rtmp.tile([128, SB], BF16, name="t1", tag="t1")
                    nc.vector.tensor_tensor(
                        t1[:], src[:, sl], cos_sb[:, sl], mybir.AluOpType.mult)
                    t2 = rtmp.tile([128, SB], BF16, name="t2", tag="t2")
                    nc.vector.tensor_tensor(
                        t2[:], rot[:], sin_sb[:, sl], mybir.AluOpType.mult)
                    nc.vector.tensor_tensor(
                        ropT_sb[e][:, sl], t1[:], t2[:], mybir.AluOpType.add)

        kT = ropT_sb[QH]
        with ExitStack() as actx:
            mpool = actx.enter_context(tc.tile_pool(name="mask", bufs=NKT + 2))
            spool = actx.enter_context(tc.tile_pool(name="sc_ps", bufs=3, space="PSUM"))
            ypool = actx.enter_context(tc.tile_pool(name="y_ps", bufs=2, space="PSUM"))
            dpool = actx.enter_context(tc.tile_pool(name="d_ps", bufs=2, space="PSUM"))
            epool = actx.enter_context(tc.tile_pool(name="exp", bufs=6))
            opool = actx.enter_context(tc.tile_pool(name="attout", bufs=4))

            for qb in range(NSB):
                qsl = slice(qb * SB, (qb + 1) * SB)
                klim = NKT
                mtiles = []
                for kt in range(klim):
                    mt = mpool.tile([128, SB], F32, name="mt", tag="mt")
                    nc.sync.dma_start(
                        out=mt[:],
                        in_=maskT[kt * 128:(kt + 1) * 128, qsl])
                    mtiles.append(mt)
                for h in range(QH):
                    ps_y = ypool.tile([HD, SB], F32, name="psy", tag="psy")
                    ps_d = dpool.tile([1, SB], F32, name="psd", tag="psd")
                    dsum = opool.tile([128, SB], F32, name="dsum", tag="dsum")
                    for kt in range(klim):
                        ps_s = spool.tile([128, SB], F32, name="pss", tag="pss")
                        nc.tensor.matmul(
                            ps_s[:],
                            lhsT=kT[:, kt * 128:(kt + 1) * 128],
                            rhs=ropT_sb[h][:, qsl],
                            start=True, stop=True)
                        et = epool.tile([128, SB], BF16, name="et", tag="et")
                        nc.vector.tensor_tensor(
                            ps_s[:], ps_s[:], mtiles[kt][:],
                            mybir.AluOpType.add)
                        nc.scalar.activation(
                            et[:], ps_s[:],
                            mybir.ActivationFunctionType.Exp, scale=SCALE)
                        nc.tensor.matmul(
                            ps_y[:],
                            lhsT=v_sb[:, kt * 128:(kt + 1) * 128],
                            rhs=et[:],
                            start=(kt == 0), stop=(kt == klim - 1))
                        if kt == 0:
                            nc.vector.tensor_copy(dsum[:], et[:])
                        else:
                            nc.vector.tensor_tensor(
                                dsum[:], dsum[:], et[:], mybir.AluOpType.add)
                    nc.tensor.matmul(
                        ps_d[:], lhsT=ones_f32[:], rhs=dsum[:],
                        start=True, stop=True)
                    recip = opool.tile([1, SB], F32, name="recip", tag="recip")
                    nc.vector.reciprocal(recip[:], ps_d[:])
                    ps_r = dpool.tile([HD, SB], F32, name="psr", tag="psr", bufs=1)
                    nc.tensor.matmul(
                        ps_r[:], lhsT=ones_row[:], rhs=recip[:],
                        start=True, stop=True)
                    rb = opool.tile([HD, SB], F32, name="rb", tag="rb")
                    nc.scalar.copy(rb[:], ps_r[:])
                    ynorm = opool.tile([HD, SB], BF16, name="ynorm", tag="ynorm")
                    nc.vector.tensor_tensor(
                        ynorm[:], ps_y[:], rb[:], mybir.AluOpType.mult)
                    nc.sync.dma_start(
                        out=ag_in[h * HD:(h + 1) * HD, qsl], in_=ynorm[:])

        nc.gpsimd.collective_compute(
            "AllGather",
            mybir.AluOpType.bypass,
            replica_groups=[list(range(NCORES))],
            ins=[ag_in[:].opt()],
            outs=[ag_out[:].opt()],
        )

        with tc.tile_pool(name="wo", bufs=1) as wpool, \
             tc.tile_pool(name="yt", bufs=NDC + 8) as ytpool, \
             tc.tile_pool(name="ops", bufs=2, space="PSUM") as opsp, \
             tc.tile_pool(name="osb", bufs=4) as osbp:
            wo_sb = []
            for dc in range(NDC):
                t = wpool.tile([128, ESH], BF16, name=f"wo{dc}", tag=f"wo{dc}")
                nc.sync.dma_start(out=t[:], in_=woT[dc * 128:(dc + 1) * 128, :])
                wo_sb.append(t)
            for sg in range(NSB):
                yts = []
                for dc in range(NDC):
                    t = ytpool.tile([128, SB], BF16, name="yt", tag="yt")
                    nc.sync.dma_start(
                        out=t[:],
                        in_=ag_out[dc * 128:(dc + 1) * 128, sg * SB:(sg + 1) * SB])
                    yts.append(t)
                for stl in range(SB // 128):
                    ps = opsp.tile([128, ESH], F32, name="ps", tag="ps")
                    for dc in range(NDC):
                        nc.tensor.matmul(
                            ps[:],
                            lhsT=yts[dc][:, stl * 128:(stl + 1) * 128],
                            rhs=wo_sb[dc][:],
                            start=(dc == 0), stop=(dc == NDC - 1))
                    ot = osbp.tile([128, ESH], F32, name="ot", tag="ot")
                    nc.scalar.copy(ot[:], ps[:])
                    st = sg * (SB // 128) + stl
                    nc.sync.dma_start(
                        out=out[st * 128:(st + 1) * 128, :], in_=ot[:])

    nc.finalize()
    return nc


_PROG_CACHE = {}


def _mask_mode_and_aux(mask):
    m = np.asarray(mask).reshape(S, S)
    if not m.any():
        return "zeros", None
    tril = np.tril(np.ones((S, S), dtype=bool))
    if (m[tril] == 0.0).all() and (m[~tril] == NEG).all():
        return "causal", None
    return "general", np.ascontiguousarray(m.T / SCALE).astype(np.float32)


def _prepare(x, mask, wqkv, wo):
    x = np.asarray(x, dtype=np.float32)
    wqkv = np.asarray(wqkv, dtype=np.float32)
    wo = np.asarray(wo, dtype=np.float32)

    mode, maskT = _mask_mode_and_aux(mask)

    xT = _bf16(x.reshape(S, D).T)                       # [D, S]
    inv = 1.0 / (10000.0 ** (np.arange(0, HD, 2, dtype=np.float32) / HD))
    t = np.arange(S, dtype=np.float32)
    freqs = np.outer(t, inv)                            # [S, HD/2]
    emb = np.concatenate([freqs, freqs], axis=-1)       # [S, HD]
    cosT = _bf16(np.cos(emb).T)                         # [HD, S]
    sinT_np = np.sin(emb).T.copy()                      # [HD, S]
    sinT_np[:HD // 2] *= -1.0                           # bake rotate_half sign
    sinT = _bf16(sinT_np)

    if mode == "causal":
        # additive pattern for diagonal tile j (k0 = qb*512 + j*128):
        # allow when q >= k, i.e. qq >= j*128 + kk  (qq, kk within tile)
        kk = np.arange(128)[:, None]
        qq = np.arange(SB)[None, :]
        pats = []
        for j in range(4):
            allow = qq >= (j * 128 + kk)
            pats.append(np.where(allow, 0.0, NEG / SCALE).astype(np.float32))
        diagp = _bf16(np.concatenate(pats, axis=1))     # [128, 2048] bf16

    in_maps = []
    for r in range(NCORES):
        q_rows = wqkv[r * QH * HD:(r + 1) * QH * HD]            # [512, D]
        k_rows = wqkv[NH * HD + r * HD: NH * HD + (r + 1) * HD]  # [128, D]
        v_rows = wqkv[(NH + NKV) * HD + r * HD:(NH + NKV) * HD + (r + 1) * HD]
        im = {
            "xT": xT,
            "wqkT": _bf16(np.concatenate([q_rows, k_rows], axis=0).T),  # [D, 640]
            "wvT": _bf16(v_rows.T),                                     # [D, 128]
            "cosT": cosT,
            "sinT": sinT,
        }
        if mode == "general":
            # column-parallel wo (post-AllGather): [D, 512] e-shard
            im["woT"] = _bf16(wo[r * ESH:(r + 1) * ESH, :].T)
        else:
            # row-parallel wo (pre-ReduceScatter): this core's 512
            # head-dims x all 4096 output cols -> [512, D]
            im["woT"] = _bf16(wo[:, r * ESH:(r + 1) * ESH].T)
        if mode == "general":
            im["maskT"] = maskT
        elif mode == "causal":
            im["diagp"] = diagp
        in_maps.append(im)
    return mode, in_maps


def kernel(x, mask, wqkv, wo):
    global LAST_RESULT
    mode, in_maps = _prepare(x, mask, wqkv, wo)

    if mode not in _PROG_CACHE:
        _PROG_CACHE[mode] = _build_program(mode)
    nc = _PROG_CACHE[mode]

    res = bass_utils.run_bass_kernel_spmd(
        nc, in_maps, core_ids=list(range(NCORES)),
        trace=bool(os.environ.get("BASS_TRACE")),
    )
    LAST_RESULT = res

    if mode == "general":
        shards = [np.asarray(res.results[r]["out"], dtype=np.float32)
                  for r in range(NCORES)]
        full = np.concatenate(shards, axis=1)           # [S, D]
    else:
        # out is [NSB, ESH, SB] bf16 = per-q-block out^T shards
        shards = []
        for r in range(NCORES):
            o = np.asarray(res.results[r]["out"], dtype=np.float32)
            shards.append(np.concatenate(list(o), axis=1).T)   # [S, ESH]
        full = np.concatenate(shards, axis=1)           # [S, D]
    return full.reshape(B, S, D)

